# revision 2
# baseline (speedup 1.0000x reference)
"""DeepSet segment-reduce kernel for 8 Trainium2 NeuronCores (Bass/Tile).

Math (reference):
    h  = relu(x1 @ W1 + b1) @ W2 + b2          # [E, 128]
    S  = segment_sum(h, seg)                   # [B, 128]
    mean = S / max(counts, 1)
    out  = mean @ W3 + b3

segment_sum is linear, so only r = relu(x1 @ W1 + b1) needs per-edge work:
segsum(h) = segsum(r) @ W2 + counts x b2.  The device computes per-PIECE
sums of r with FUSED relu+accumulate instructions; everything downstream
(piece->segment combine, the tiny W2/W3 stage) runs on the host (0.2% of
the FLOPs).

Device pipeline per core (one SPMD program on 8 cores):
  - xT [128, e_cap] bf16 streamed from DRAM (big DMA tiles)
  - hT = W1.T @ xT + b1 on PE (bf16 matmul + rank-1 b1 add) -> PSUM fp32
  - per piece (a run of columns belonging to one segment):
    ONE fused instruction computing relu -> scratch and sum -> accum slot:
      ACT:  activation(Relu, accum_out=slot)
      DVE:  tensor_scalar(max 0, op1=add reduce, accum_out=slot)
    pieces are split between ACT and DVE by a cost model so both engines
    finish together (~2x faster than the relu-pass + reduce-pass split).
  - accum [128, S] fp32 DMA'd out at the end.

SPMD uniformity: piece lengths are baked into the instruction stream, so
all 8 cores must share one schedule. The host pads each segment to a
multiple of 128 cols (zero columns; their relu(b1) contribution is
subtracted on the host), chops segments into pieces <= 2048 (PSUM tile),
then splits pieces until each piece-length count is divisible by 8, so
every core gets an identical multiset of piece lengths. Any piece may go
to any core; the host epilogue recombines by segment id.

Self-contained: no reads of /root/problem/*; shapes derived from inputs.
"""

import numpy as np

N_CORES = 8
BLOCK = 128          # segment padding quantum (cols)
PIECE_MAX = 2048     # max piece length == PSUM tile cols (4 banks fp32)
PSUM_TILE = 2048
DMA_TILE = 4096      # xT cols per DMA (bf16 -> 1 MiB)

# per-piece engine cost model (ns), for ACT/DVE load balancing
ACT_FIX, ACT_PER = 330.0, 1.0 / 1.2
DVE_FIX, DVE_PER = 125.0, 1.0 / 0.96


def _bf16():
    import ml_dtypes
    return ml_dtypes.bfloat16


def _plan(edge_slices, E, B):
    """Build the universal piece schedule + per-core piece assignment."""
    es = np.asarray(edge_slices, dtype=np.int64)
    counts = (es[1:] - es[:-1]).astype(np.int64)

    # global pieces: (seg, start_col_in_padded_segment, length)
    by_len = {L: [] for L in range(BLOCK, PIECE_MAX + 1, BLOCK)}
    for b in range(B):
        c = int(counts[b])
        if c == 0:
            continue
        total = ((c + BLOCK - 1) // BLOCK) * BLOCK
        start = 0
        while total > 0:
            L = min(total, PIECE_MAX)
            by_len[L].append((b, start, L))
            start += L
            total -= L

    # make every length-count divisible by N_CORES by splitting pieces
    for L in range(PIECE_MAX, BLOCK, -BLOCK):
        lst = by_len[L]
        for _ in range(len(lst) % N_CORES):
            seg, st, _ = lst.pop()
            nb = L // BLOCK
            L1 = (nb - nb // 2) * BLOCK
            L2 = L - L1
            by_len[L1].append((seg, st, L1))
            by_len[L2].append((seg, st + L1, L2))
    ndum = (-len(by_len[BLOCK])) % N_CORES
    for _ in range(ndum):
        by_len[BLOCK].append((-1, 0, BLOCK))  # dummy (all-zero cols)

    # universal per-core multiset: k_L pieces of each length L
    per_core_k = {L: len(by_len[L]) // N_CORES for L in by_len}

    # bin-pack the per-core multiset into PSUM tiles of PIECE_MAX cols
    # (first-fit decreasing, exact-fit preferred; gaps stay un-accum'd)
    items = []  # lengths, descending
    for L in range(PIECE_MAX, BLOCK - 1, -BLOCK):
        items += [L] * per_core_k[L]
    bins = []  # list of (list_of_lengths, free)
    for L in items:
        for b_ in bins:
            if b_[1] >= L:
                b_[0].append(L)
                b_[1] -= L
                break
        else:
            bins.append([[L], PIECE_MAX - L])
    n_bins = len(bins)
    if n_bins % (DMA_TILE // PSUM_TILE):
        bins.append([[], PIECE_MAX])
        n_bins += 1
    e_cap = n_bins * PSUM_TILE

    # schedule: per tile, (offset, fd, engine, slot); slots numbered in
    # schedule order; engine by greedy balance (largest fd first)
    entries = []  # (tile, off, fd)
    for t, (lens, _) in enumerate(bins):
        off = 0
        for L in lens:
            entries.append([t, off, L])
            off += L
    order = sorted(range(len(entries)), key=lambda i: -entries[i][2])
    eng = [0] * len(entries)
    tA = tD = 0.0
    for i in order:
        fd = entries[i][2]
        cA = ACT_FIX + ACT_PER * fd
        cD = DVE_FIX + DVE_PER * fd
        if tA + cA <= tD + cD:
            eng[i] = 0
            tA += cA
        else:
            eng[i] = 1
            tD += cD
    sched = []  # (tile, off, fd, engine, slot)
    for slot, (e_, i) in enumerate(zip(eng, range(len(entries)))):
        t, off, fd = entries[i]
        sched.append((t, off, fd, e_, slot))
    n_slots = len(sched)

    # per-core piece assignment: core c takes the c-th slice of each
    # length list; its i-th piece of length L fills its i-th schedule
    # slot of length L
    slots_by_len = {}
    for (t, off, fd, e_, slot) in sched:
        slots_by_len.setdefault(fd, []).append(slot)
    core_pieces = []  # [core][slot] -> (seg, start, fd) or None
    for c in range(N_CORES):
        pieces = [None] * n_slots
        for L, slots in slots_by_len.items():
            k = per_core_k[L]
            mine = by_len[L][c * k:(c + 1) * k]
            for s_, p_ in zip(slots, mine):
                if p_[0] >= 0:
                    pieces[s_] = p_
        core_pieces.append(pieces)

    return {
        "es": es, "counts": counts, "e_cap": e_cap, "sched": sched,
        "n_slots": n_slots, "core_pieces": core_pieces, "B": B,
    }


def _build_core_inputs(x1, plan):
    bf16 = _bf16()
    es = plan["es"]
    e_cap = plan["e_cap"]
    sched = plan["sched"]
    xT = np.ascontiguousarray(x1.T).astype(bf16)  # [128, E]
    xTs, n_pads = [], []
    slot_pos = {s: (t * PSUM_TILE + off, fd) for (t, off, fd, e_, s) in sched}
    for c in range(N_CORES):
        xc = np.zeros((128, e_cap), dtype=bf16)
        npad = np.zeros(plan["n_slots"], dtype=np.int64)
        for s, piece in enumerate(plan["core_pieces"][c]):
            col0, fd = slot_pos[s]
            if piece is None:
                npad[s] = 0  # dummy: zero cols, but host ignores slot
                continue
            seg, st, L = piece
            a = es[seg] + st
            real = min(L, int(es[seg + 1] - a))
            if real > 0:
                xc[:, col0:col0 + real] = xT[:, a:a + real]
            npad[s] = L - max(real, 0)
        xTs.append(xc)
        n_pads.append(npad)
    return xTs, n_pads


def _build_bass(e_cap, sched, n_slots):
    import concourse.bacc as bacc
    import concourse.mybir as mybir
    import concourse.tile as tile

    f32 = mybir.dt.float32
    bf = mybir.dt.bfloat16
    Relu = mybir.ActivationFunctionType.Relu
    Max = mybir.AluOpType.max
    Add = mybir.AluOpType.add

    nc = bacc.Bacc(trn_type="TRN2", num_devices=N_CORES)

    xT_d = nc.dram_tensor("xT", [128, e_cap], bf, kind="ExternalInput")
    W1_d = nc.dram_tensor("W1b", [128, 128], bf, kind="ExternalInput")
    b1_d = nc.dram_tensor("b1r", [1, 128], bf, kind="ExternalInput")
    acc_d = nc.dram_tensor("acc", [128, n_slots], f32, kind="ExternalOutput")

    n_dma = e_cap // DMA_TILE
    per_dma = DMA_TILE // PSUM_TILE

    sched_by_tile = {}
    for (t, off, fd, e_, slot) in sched:
        sched_by_tile.setdefault(t, []).append((off, fd, e_, slot))

    with tile.TileContext(nc) as tc, tc.tile_pool(name="persist", bufs=1) as pp:
        w1_sb = pp.tile([128, 128], bf, name="w1_sb")
        b1_sb = pp.tile([1, 128], bf, name="b1_sb")
        ones_sb = pp.tile([1, 512], bf, name="ones_sb")
        acc_sb = pp.tile([128, n_slots], f32, name="acc_sb")
        nc.sync.dma_start(w1_sb[:], W1_d[:])
        nc.sync.dma_start(b1_sb[:], b1_d[:])
        nc.vector.memset(ones_sb[:], 1.0)

        with (
            tc.tile_pool(name="xp", bufs=3) as xp,
            tc.tile_pool(name="hp", bufs=2, space="PSUM") as hp,
            tc.tile_pool(name="sa", bufs=2) as sa,
            tc.tile_pool(name="sd", bufs=2) as sd,
        ):
            for t in range(n_dma):
                xt = xp.tile([128, DMA_TILE], bf, name="xt")
                nc.sync.dma_start(
                    xt[:], xT_d[:, t * DMA_TILE:(t + 1) * DMA_TILE])
                for h in range(per_dma):
                    tile_idx = t * per_dma + h
                    ps = hp.tile([128, PSUM_TILE], f32, name="ps")
                    for q in range(PSUM_TILE // 512):
                        c0 = h * PSUM_TILE + q * 512
                        sl = slice(q * 512, (q + 1) * 512)
                        nc.tensor.matmul(
                            ps[:, sl], lhsT=w1_sb[:], rhs=xt[:, c0:c0 + 512],
                            start=True, stop=False)
                        nc.tensor.matmul(
                            ps[:, sl], lhsT=b1_sb[0:1, :],
                            rhs=ones_sb[0:1, :], start=False, stop=True)
                    for (off, fd, e_, slot) in sched_by_tile.get(tile_idx, []):
                        acc_ap = acc_sb[:, slot:slot + 1]
                        if e_ == 0:
                            sc = sa.tile([128, PSUM_TILE], bf, name="sca")
                            nc.scalar.activation(
                                sc[:, :fd], ps[:, off:off + fd], Relu,
                                bias=0.0, accum_out=acc_ap)
                        else:
                            sc = sd.tile([128, PSUM_TILE], bf, name="scd")
                            nc.vector.tensor_scalar(
                                sc[:, :fd], ps[:, off:off + fd], 0.0, None,
                                op0=Max, op1=Add, accum_out=acc_ap)

        nc.sync.dma_start(acc_d[:], acc_sb[:])

    nc.compile()
    return nc


def _prepare(x1, edge_slices, W1, b1, W2, b2, W3, b3):
    bf16 = _bf16()
    x1 = np.ascontiguousarray(np.asarray(x1, dtype=np.float32))
    E = x1.shape[0]
    B = int(np.asarray(edge_slices).shape[0]) - 1

    plan = _plan(edge_slices, E, B)
    xTs, n_pads = _build_core_inputs(x1, plan)
    plan["n_pads"] = n_pads

    W1b = np.asarray(W1, np.float32).astype(bf16)
    b1b = np.asarray(b1, np.float32).astype(bf16).reshape(1, 128)
    shared = {"W1b": np.ascontiguousarray(W1b),
              "b1r": np.ascontiguousarray(b1b)}

    nc = _build_bass(plan["e_cap"], plan["sched"], plan["n_slots"])
    in_maps = [{"xT": xTs[c], **shared} for c in range(N_CORES)]
    return nc, in_maps, plan


def _finish(acc_list, plan, b1, W2, b2, W3, b3):
    """Host epilogue: piece sums -> segment sums -> mean -> W3."""
    bf16 = _bf16()
    B = plan["B"]
    counts = plan["counts"].astype(np.float32)
    relu_b1 = np.maximum(
        np.asarray(b1, np.float32).astype(bf16).astype(np.float32), 0.0)

    R = np.zeros((B, 128), dtype=np.float64)
    for c in range(N_CORES):
        acc = np.asarray(acc_list[c], np.float64)  # [128, S]
        npad = plan["n_pads"][c]
        for s, piece in enumerate(plan["core_pieces"][c]):
            if piece is None:
                continue
            seg = piece[0]
            R[seg] += acc[:, s]
            if npad[s]:
                R[seg] -= npad[s] * relu_b1
    R = R.astype(np.float32)

    W2 = np.asarray(W2, np.float32)
    b2 = np.asarray(b2, np.float32)
    W3 = np.asarray(W3, np.float32)
    b3 = np.asarray(b3, np.float32)
    sums_h = R @ W2 + counts[:, None] * b2[None, :]
    mean = sums_h / np.maximum(counts, 1.0)[:, None]
    return (mean @ W3 + b3[None, :]).astype(np.float32)


def kernel(x1, edge_slices, W1, b1, W2, b2, W3, b3):
    from concourse import bass_utils

    nc, in_maps, plan = _prepare(x1, edge_slices, W1, b1, W2, b2, W3, b3)
    br = bass_utils.run_bass_kernel_spmd(
        nc, in_maps, core_ids=list(range(N_CORES)))
    return _finish([r["acc"] for r in br.results], plan, b1, W2, b2, W3, b3)


# revision 10
# speedup vs baseline: 1.6713x; 1.6713x over previous
"""DeepSet segment-reduce kernel for 8 Trainium2 NeuronCores (Bass/Tile).

Math (reference):
    h  = relu(x1 @ W1 + b1) @ W2 + b2          # [E, 128]
    S  = segment_sum(h, seg)                   # [B, 128]
    mean = S / max(counts, 1)
    out  = mean @ W3 + b3

segment_sum is linear, so only r = relu(x1 @ W1 + b1) needs per-edge work:
segsum(h) = segsum(r) @ W2 + counts x b2.  The device computes per-PIECE
sums of r with FUSED relu+accumulate instructions; everything downstream
(piece->segment combine, the tiny W2/W3 stage) runs on the host (0.2% of
the FLOPs).

Device pipeline per core (one SPMD program on 8 cores):
  - xT [128, e_cap] bf16 streamed from DRAM (big DMA tiles)
  - hT = W1.T @ xT + b1 on PE (bf16 matmul + rank-1 b1 add) -> PSUM fp32
  - per piece (a run of columns belonging to one segment):
    ONE fused instruction computing relu -> scratch and sum -> accum slot:
      ACT:  activation(Relu, accum_out=slot)
      DVE:  tensor_scalar(max 0, op1=add reduce, accum_out=slot)
    pieces are split between ACT and DVE by a cost model so both engines
    finish together (~2x faster than the relu-pass + reduce-pass split).
  - accum [128, S] fp32 DMA'd out at the end.

SPMD uniformity: piece lengths are baked into the instruction stream, so
all 8 cores must share one schedule. The host pads each segment to a
multiple of 128 cols (zero columns; their relu(b1) contribution is
subtracted on the host), chops segments into pieces <= 2048 (PSUM tile),
then splits pieces until each piece-length count is divisible by 8, so
every core gets an identical multiset of piece lengths. Any piece may go
to any core; the host epilogue recombines by segment id.

Self-contained: no reads of /root/problem/*; shapes derived from inputs.
"""

import numpy as np

N_CORES = 8
BLOCK = 128          # segment padding quantum (cols)
PIECE_MAX = 2048     # max piece length == PSUM tile cols (4 banks fp32)
PSUM_TILE = 2048
DMA_TILE = 8192      # xT cols per DMA (bf16 -> 2 MiB, 16 KB/partition)

# per-piece engine cost model (ns), for ACT/DVE load balancing
ACT_FIX, ACT_PER = 330.0, 1.0 / 1.2
DVE_FIX, DVE_PER = 125.0, 1.0 / 0.96


def _bf16():
    import ml_dtypes
    return ml_dtypes.bfloat16


def _plan(edge_slices, E, B):
    """Build the universal piece schedule + per-core piece assignment."""
    es = np.asarray(edge_slices, dtype=np.int64)
    counts = (es[1:] - es[:-1]).astype(np.int64)

    # global pieces: (seg, start_col_in_padded_segment, length)
    by_len = {L: [] for L in range(BLOCK, PIECE_MAX + 1, BLOCK)}
    for b in range(B):
        c = int(counts[b])
        if c == 0:
            continue
        total = ((c + BLOCK - 1) // BLOCK) * BLOCK
        start = 0
        while total > 0:
            L = min(total, PIECE_MAX)
            by_len[L].append((b, start, L))
            start += L
            total -= L

    # make every length-count divisible by N_CORES by splitting pieces
    for L in range(PIECE_MAX, BLOCK, -BLOCK):
        lst = by_len[L]
        for _ in range(len(lst) % N_CORES):
            seg, st, _ = lst.pop()
            nb = L // BLOCK
            L1 = (nb - nb // 2) * BLOCK
            L2 = L - L1
            by_len[L1].append((seg, st, L1))
            by_len[L2].append((seg, st + L1, L2))
    ndum = (-len(by_len[BLOCK])) % N_CORES
    for _ in range(ndum):
        by_len[BLOCK].append((-1, 0, BLOCK))  # dummy (all-zero cols)

    # universal per-core multiset: k_L pieces of each length L
    per_core_k = {L: len(by_len[L]) // N_CORES for L in by_len}

    # bin-pack the per-core multiset into PSUM tiles of PIECE_MAX cols
    # (first-fit decreasing, exact-fit preferred; gaps stay un-accum'd)
    items = []  # lengths, descending
    for L in range(PIECE_MAX, BLOCK - 1, -BLOCK):
        items += [L] * per_core_k[L]
    bins = []  # list of (list_of_lengths, free)
    for L in items:
        for b_ in bins:
            if b_[1] >= L:
                b_[0].append(L)
                b_[1] -= L
                break
        else:
            bins.append([[L], PIECE_MAX - L])
    n_bins = len(bins)
    while n_bins % (DMA_TILE // PSUM_TILE):
        bins.append([[], PIECE_MAX])
        n_bins += 1
    e_cap = n_bins * PSUM_TILE

    # schedule: per tile, (offset, fd, engine, slot); slots numbered in
    # schedule order; engine by greedy balance (largest fd first)
    entries = []  # (tile, off, fd)
    for t, (lens, _) in enumerate(bins):
        off = 0
        for L in lens:
            entries.append([t, off, L])
            off += L
    order = sorted(range(len(entries)), key=lambda i: -entries[i][2])
    eng = [0] * len(entries)
    tA = tD = 0.0
    for i in order:
        fd = entries[i][2]
        cA = ACT_FIX + ACT_PER * fd
        cD = DVE_FIX + DVE_PER * fd
        if tA + cA <= tD + cD:
            eng[i] = 0
            tA += cA
        else:
            eng[i] = 1
            tD += cD
    sched = []  # (tile, off, fd, engine, slot)
    for slot, (e_, i) in enumerate(zip(eng, range(len(entries)))):
        t, off, fd = entries[i]
        sched.append((t, off, fd, e_, slot))
    n_slots = len(sched)

    # per-core piece assignment: core c takes the c-th slice of each
    # length list; its i-th piece of length L fills its i-th schedule
    # slot of length L
    slots_by_len = {}
    for (t, off, fd, e_, slot) in sched:
        slots_by_len.setdefault(fd, []).append(slot)
    core_pieces = []  # [core][slot] -> (seg, start, fd) or None
    for c in range(N_CORES):
        pieces = [None] * n_slots
        for L, slots in slots_by_len.items():
            k = per_core_k[L]
            mine = by_len[L][c * k:(c + 1) * k]
            for s_, p_ in zip(slots, mine):
                if p_[0] >= 0:
                    pieces[s_] = p_
        core_pieces.append(pieces)

    return {
        "es": es, "counts": counts, "e_cap": e_cap, "sched": sched,
        "n_slots": n_slots, "core_pieces": core_pieces, "B": B,
    }


def _build_core_inputs(x1, plan):
    bf16 = _bf16()
    es = plan["es"]
    e_cap = plan["e_cap"]
    sched = plan["sched"]
    xT = np.ascontiguousarray(x1.T).astype(bf16)  # [128, E]
    xTs, n_pads = [], []
    slot_pos = {s: (t * PSUM_TILE + off, fd) for (t, off, fd, e_, s) in sched}
    for c in range(N_CORES):
        xc = np.zeros((128, e_cap), dtype=bf16)
        npad = np.zeros(plan["n_slots"], dtype=np.int64)
        for s, piece in enumerate(plan["core_pieces"][c]):
            col0, fd = slot_pos[s]
            if piece is None:
                npad[s] = 0  # dummy: zero cols, but host ignores slot
                continue
            seg, st, L = piece
            a = es[seg] + st
            real = min(L, int(es[seg + 1] - a))
            if real > 0:
                xc[:, col0:col0 + real] = xT[:, a:a + real]
            npad[s] = L - max(real, 0)
        xTs.append(xc)
        n_pads.append(npad)
    return xTs, n_pads


def _build_bass(e_cap, sched, n_slots):
    import concourse.bacc as bacc
    import concourse.mybir as mybir
    import concourse.tile as tile

    f32 = mybir.dt.float32
    bf = mybir.dt.bfloat16
    Relu = mybir.ActivationFunctionType.Relu
    Max = mybir.AluOpType.max
    Add = mybir.AluOpType.add

    nc = bacc.Bacc(trn_type="TRN2", num_devices=N_CORES)

    xT_d = nc.dram_tensor("xT", [128, e_cap], bf, kind="ExternalInput")
    W1_d = nc.dram_tensor("W1b", [128, 128], bf, kind="ExternalInput")
    b1_d = nc.dram_tensor("b1c", [128, 1], f32, kind="ExternalInput")
    acc_d = nc.dram_tensor("acc", [128, n_slots], f32, kind="ExternalOutput")

    n_dma = e_cap // DMA_TILE
    per_dma = DMA_TILE // PSUM_TILE

    sched_by_tile = {}
    for (t, off, fd, e_, slot) in sched:
        sched_by_tile.setdefault(t, []).append((off, fd, e_, slot))

    with tile.TileContext(nc) as tc, tc.tile_pool(name="persist", bufs=1) as pp:
        w1_sb = pp.tile([128, 128], bf, name="w1_sb")
        b1_sb = pp.tile([128, 1], f32, name="b1_sb")
        zero_sb = pp.tile([128, PSUM_TILE], f32, name="zero_sb")
        acc_sb = pp.tile([128, n_slots], f32, name="acc_sb")
        nc.sync.dma_start(w1_sb[:], W1_d[:])
        nc.sync.dma_start(b1_sb[:], b1_d[:])
        nc.vector.memset(zero_sb[:], 0.0)

        with (
            tc.tile_pool(name="xp", bufs=3) as xp,
            tc.tile_pool(name="hp", bufs=2, space="PSUM") as hp,
            tc.tile_pool(name="sa", bufs=2) as sa,
            tc.tile_pool(name="sd", bufs=2) as sd,
        ):
            for t in range(n_dma):
                xt = xp.tile([128, DMA_TILE], bf, name="xt")
                nc.sync.dma_start(
                    xt[:], xT_d[:, t * DMA_TILE:(t + 1) * DMA_TILE])
                for h in range(per_dma):
                    tile_idx = t * per_dma + h
                    ps = hp.tile([128, PSUM_TILE], f32, name="ps")
                    if tile_idx == 0:
                        # HAM warm-up: ~4.5us of back-to-back matmuls so
                        # the PE clock gate opens (K=8/8) before the real
                        # pipeline starts; the first real matmul's
                        # start=True overwrites this garbage.
                        for _ in range(42):
                            nc.tensor.matmul(ps[:, 0:128], lhsT=w1_sb[:],
                                             rhs=w1_sb[:],
                                             start=True, stop=True)
                    for q in range(PSUM_TILE // 512):
                        c0 = h * PSUM_TILE + q * 512
                        sl = slice(q * 512, (q + 1) * 512)
                        nc.tensor.matmul(
                            ps[:, sl], lhsT=w1_sb[:], rhs=xt[:, c0:c0 + 512],
                            start=True, stop=True)
                    for (off, fd, e_, slot) in sched_by_tile.get(tile_idx, []):
                        acc_ap = acc_sb[:, slot:slot + 1]
                        if e_ == 0:
                            sc = sa.tile([128, PSUM_TILE], bf, name="sca")
                            nc.scalar.activation(
                                sc[:, :fd], ps[:, off:off + fd], Relu,
                                bias=b1_sb[:, 0:1], accum_out=acc_ap)
                        else:
                            sc = sd.tile([128, PSUM_TILE], bf, name="scd")
                            nc.vector.scalar_tensor_tensor(
                                sc[:, :fd], ps[:, off:off + fd],
                                b1_sb[:, 0:1], zero_sb[:, :fd],
                                op0=Add, op1=Max, accum_out=acc_ap)

        nc.sync.dma_start(acc_d[:], acc_sb[:])

    nc.compile()
    return nc


def _prepare(x1, edge_slices, W1, b1, W2, b2, W3, b3):
    bf16 = _bf16()
    x1 = np.ascontiguousarray(np.asarray(x1, dtype=np.float32))
    E = x1.shape[0]
    B = int(np.asarray(edge_slices).shape[0]) - 1

    plan = _plan(edge_slices, E, B)
    xTs, n_pads = _build_core_inputs(x1, plan)
    plan["n_pads"] = n_pads

    W1b = np.asarray(W1, np.float32).astype(bf16)
    b1c = np.asarray(b1, np.float32).reshape(128, 1)
    shared = {"W1b": np.ascontiguousarray(W1b),
              "b1c": np.ascontiguousarray(b1c)}

    nc = _build_bass(plan["e_cap"], plan["sched"], plan["n_slots"])
    in_maps = [{"xT": xTs[c], **shared} for c in range(N_CORES)]
    return nc, in_maps, plan


def _finish(acc_list, plan, b1, W2, b2, W3, b3):
    """Host epilogue: piece sums -> segment sums -> mean -> W3."""
    B = plan["B"]
    counts = plan["counts"].astype(np.float32)
    # zero-pad cols produce relu(0 @ W1 + b1) = relu(b1): subtract on host
    relu_b1 = np.maximum(np.asarray(b1, np.float32), 0.0)

    R = np.zeros((B, 128), dtype=np.float64)
    for c in range(N_CORES):
        acc = np.asarray(acc_list[c], np.float64)  # [128, S]
        npad = plan["n_pads"][c]
        for s, piece in enumerate(plan["core_pieces"][c]):
            if piece is None:
                continue
            seg = piece[0]
            R[seg] += acc[:, s]
            if npad[s]:
                R[seg] -= npad[s] * relu_b1
    R = R.astype(np.float32)

    W2 = np.asarray(W2, np.float32)
    b2 = np.asarray(b2, np.float32)
    W3 = np.asarray(W3, np.float32)
    b3 = np.asarray(b3, np.float32)
    sums_h = R @ W2 + counts[:, None] * b2[None, :]
    mean = sums_h / np.maximum(counts, 1.0)[:, None]
    return (mean @ W3 + b3[None, :]).astype(np.float32)


def kernel(x1, edge_slices, W1, b1, W2, b2, W3, b3):
    from concourse import bass_utils

    nc, in_maps, plan = _prepare(x1, edge_slices, W1, b1, W2, b2, W3, b3)
    br = bass_utils.run_bass_kernel_spmd(
        nc, in_maps, core_ids=list(range(N_CORES)))
    return _finish([r["acc"] for r in br.results], plan, b1, W2, b2, W3, b3)
